# revision 12
# baseline (speedup 1.0000x reference)
"""Trainium2 Bass kernel for nn_Event_Critic_Net (dual-branch GAT critic).

Math: the reference reads the GAT output only at the LAST node of each
graph (graphs are 32 contiguous nodes), so only edges with dst == last
node contribute.  Per graph g:

    out_g = sigmoid( (sum_n alpha[n] x[n]) @ W + bias )
    alpha[n] = cnt[n] exp(e[n]) / (sum + 1e-16)
    e[n] = leaky_relu(x[n].w_src + x[last(g)].w_dst)

Only nodes with cnt>0 matter (~7 of 32 per graph), so the host GATHERS
contributing nodes and bin-packs graphs into 128-slot tiles, T=36 tiles
x C=20 graph-columns per core per branch.  Per-slot logits come from a
single matmul per tile: the feature-major tensor stacks the slot's own
features (rows 0:64) on top of its graph's last-node features (rows
64:128), so lhsT.T @ [w_src; w_dst] yields e directly.  All scalar ops
(Prelu, Exp, Copy, Tanh) live in ONE activation table; sigmoid is
tanh-based with the algebra folded into the MLP head host-side:
    sig_u*sig_d*mlpW = mlpW/4 . (1 + tu + (1+tu)*td).
The cnt-mask ships as fp8e4 bytes packed inside the bf16 stream.
Projection runs on UNNORMALIZED aggregates while the denominator
reciprocal broadcast proceeds in parallel (scalar copy -> PE ones
broadcast -> 128-lane reciprocal); one multiply then feeds tanh.
Each branch chunk ships as ONE concatenated HWDGE DMA, u before d,
queues byte-balanced.  Graphs are data-parallel over 8 cores.
"""

import numpy as np
from contextlib import ExitStack

NC = 8            # cores
N = 131072        # nodes total
G = 4096          # graphs
NPG = 32          # nodes per graph
S = 64            # state size
H = 128           # hidden size
GPC = G // NC     # 512 graphs per core
T = 36            # slot tiles per branch per core
C = 20            # graph columns per tile
TC = T * C        # 720 output columns per core
HT = T // 2       # tiles per chunk (2 chunks)
NEG = 0.2

XGT2W = HT * 128  # 2304
XGW = HT * 64     # 1152
CMW = HT * C      # 360 fp8 bytes -> 180 bf16 cols
CMB = CMW // 2    # 180
CHW = XGT2W + XGW + CMB  # 3636 cols per chunk

_CACHE = {}


def _build_module():
    import concourse.tile as tile
    from concourse import bacc, mybir
    from concourse.alu_op_type import AluOpType as Alu

    f32 = mybir.dt.float32
    bf16 = mybir.dt.bfloat16
    fp8 = mybir.dt.float8e4
    Act = mybir.ActivationFunctionType

    nc = bacc.Bacc("TRN2", target_bir_lowering=False, debug=False,
                   num_devices=NC)
    obias_f = float(_CACHE["obias"])

    dram = {}
    for p in ("u", "d"):
        dram[p] = nc.dram_tensor(f"{p}_dat", [128, 2 * CHW], bf16,
                                 kind="ExternalInput")
    dram["cstf"] = nc.dram_tensor("cstf", [128, 140], f32,
                                  kind="ExternalInput")
    dram["cstb"] = nc.dram_tensor("cstb", [128, 261], bf16,
                                  kind="ExternalInput")
    out_dram = nc.dram_tensor("out", [1, TC], f32, kind="ExternalOutput")

    with tile.TileContext(nc) as tc, ExitStack() as ctx:
        const = ctx.enter_context(tc.tile_pool(name="const", bufs=1))
        xp = ctx.enter_context(tc.tile_pool(name="xp", bufs=2))
        wk = ctx.enter_context(tc.tile_pool(name="wk", bufs=2))
        pse = ctx.enter_context(tc.tile_pool(name="pse", bufs=1,
                                             space="PSUM"))
        psy = ctx.enter_context(tc.tile_pool(name="psy", bufs=2,
                                             space="PSUM"))
        psr = ctx.enter_context(tc.tile_pool(name="psr", bufs=2,
                                             space="PSUM"))
        psh = ctx.enter_context(tc.tile_pool(name="psh", bufs=2,
                                             space="PSUM"))

        cstb = const.tile([128, 261], bf16, tag="cstb")
        nc.gpsimd.dma_start(cstb[:], dram["cstb"].ap())
        cstf = const.tile([128, 140], f32, tag="cstf")
        nc.gpsimd.dma_start(cstf[:], dram["cstf"].ap())
        hbiases = {"u": cstf[:, 6:7], "d": cstf[:, 7:8]}   # bias/2
        onesrow = cstf[0:1, 8:136]                          # [1,128] ones
        Ws = {"u": cstb[0:64, 0:128], "d": cstb[0:64, 128:256]}
        mlpW4 = cstb[:, 256:257]                            # mlp_W/4
        w2s = {"u": cstb[:, 257:258], "d": cstb[:, 258:259]}
        ones128 = cstb[:, 260:261]

        st = {"u": {}, "d": {}}
        for p in ("u", "d"):
            st[p]["ch"] = [None, None]
        # byte-balanced queues, earliest-needed first
        for eng, loads in ((nc.sync, (("u", 0), ("d", 0))),
                           (nc.scalar, (("u", 1), ("d", 1)))):
            for p, c in loads:
                t = xp.tile([128, CHW], bf16, tag=f"ch{c}",
                            name=f"ch{c}_{p}")
                eng.dma_start(t[:],
                              dram[p].ap()[:, c * CHW:(c + 1) * CHW])
                st[p]["ch"][c] = t

        eraw = pse.tile([128, 2 * T], f32, tag="eraw")
        tanh = {}
        for p in ("u", "d"):
            st[p]["ysb"] = wk.tile([64, TC], bf16, tag="ysb",
                                   name=f"ysb_{p}")
            st[p]["dnS"] = wk.tile([1, TC], f32, tag="dnS",
                                   name=f"dnS_{p}")
            st[p]["hn"] = wk.tile([128, TC], bf16, tag="hn",
                                  name=f"hn_{p}")
            tanh[p] = wk.tile([128, TC], bf16, tag="th", name=f"th_{p}")

        def logits_masks_agg(p, c):
            """eraw -> prelu/exp -> mask -> agg + denom for chunk c."""
            s = st[p]
            off = (0 if p == "u" else T) + c * HT
            ch = s["ch"][c]
            for t in range(HT):
                nc.tensor.matmul(
                    eraw[:, off + t: off + t + 1],
                    ch[:, 128 * t:128 * t + 128],
                    w2s[p], start=True, stop=True)
            e = wk.tile([128, HT], f32, tag="e", name=f"e_{p}{c}")
            nc.scalar.activation(e[:], eraw[:, off:off + HT], Act.Prelu,
                                 alpha=NEG)
            ex = wk.tile([128, HT], bf16, tag="ex", name=f"ex_{p}{c}")
            nc.scalar.activation(ex[:], e[:], Act.Exp)
            M = wk.tile([128, CMW], bf16, tag="M", name=f"M_{p}{c}")
            Mv = M[:].rearrange("p (t c) -> p t c", c=C)
            cmv = ch[:, XGT2W + XGW:CHW].bitcast(fp8).rearrange(
                "p (t c) -> p t c", c=C)
            exb = ex[:][:, :, None].broadcast_to([128, HT, C])
            nc.vector.tensor_tensor(Mv, exb, cmv, op=Alu.mult)
            ynT = psy.tile([128, CMW], f32, tag="ynT", name=f"ynT_{p}{c}")
            # denominator subchain first: hides behind agg + projection
            nc.tensor.matmul(ynT[64:65, :], ones128, M[:],
                             start=True, stop=True)
            nc.scalar.activation(s["dnS"][:, c * CMW:(c + 1) * CMW],
                                 ynT[64:65, :], Act.Copy, bias=1e-16)
            for t in range(HT):
                nc.tensor.matmul(
                    ynT[0:64, C * t:C * (t + 1)],
                    ch[:, XGT2W + 64 * t:XGT2W + 64 * t + 64],
                    M[:, C * t:C * (t + 1)], start=True, stop=True)
            s.setdefault("ynT", []).append(ynT)

        def tail_half(p, c):
            """denominator path (dnS -> PE bcast -> recip) runs beside
            the main path (ysb bf16 -> raw proj); then hn -> tanh."""
            s = st[p]
            cs = slice(c * CMW, (c + 1) * CMW)
            nc.vector.tensor_copy(s["ysb"][:, cs], s["ynT"][c][0:64, :])
            rbc = psr.tile([128, CMW], f32, tag="rbc", name=f"rbc_{p}{c}")
            nc.tensor.matmul(rbc[:], onesrow, s["dnS"][:, cs],
                             start=True, stop=True)
            rc = wk.tile([128, CMW], f32, tag="rc", name=f"rc_{p}{c}")
            nc.vector.reciprocal_approx_fast(rc[:], rbc[:])
            hraw = psh.tile([128, CMW], f32, tag="hraw",
                            name=f"hraw_{p}{c}")
            nc.tensor.matmul(hraw[:], Ws[p], s["ysb"][:, cs],
                             start=True, stop=True)
            nc.vector.tensor_tensor(s["hn"][:, cs], hraw[:], rc[:],
                                    op=Alu.mult)
            nc.scalar.activation(tanh[p][:, cs], s["hn"][:, cs], Act.Tanh,
                                 bias=hbiases[p], scale=0.5)

        def head_half(c):
            """o = obias + mlpW/4 . (tu + (1+tu)*td) per half."""
            cs = slice(c * CMW, (c + 1) * CMW)
            q = wk.tile([128, CMW], bf16, tag="q", name=f"q{c}")
            nc.vector.scalar_tensor_tensor(q[:], tanh["u"][:, cs], 1.0,
                                           tanh["d"][:, cs], op0=Alu.add,
                                           op1=Alu.mult)
            o_ps = pse.tile([1, CMW], f32, tag="mix", name=f"o{c}")
            nc.tensor.matmul(o_ps[:], mlpW4, tanh["u"][:, cs],
                             start=True, stop=False)
            nc.tensor.matmul(o_ps[:], mlpW4, q[:],
                             start=False, stop=True)
            o_sb = st["u"]["o_sb"]
            nc.scalar.activation(o_sb[:, cs], o_ps[:], Act.Copy,
                                 bias=obias_f)
            nc.scalar.dma_start(out_dram.ap()[:, cs], o_sb[:, cs])

        st["u"]["o_sb"] = wk.tile([1, TC], f32, tag="o_sb", name="o_sb")

        logits_masks_agg("u", 0)
        logits_masks_agg("u", 1)
        tail_half("u", 0)
        tail_half("u", 1)
        logits_masks_agg("d", 0)
        tail_half("d", 0)
        logits_masks_agg("d", 1)
        head_half(0)
        tail_half("d", 1)
        head_half(1)

    nc.compile()
    return nc


def _get_module():
    if "nc" not in _CACHE:
        _CACHE["nc"] = _build_module()
    return _CACHE["nc"]


def _pack_core(szs):
    """First-fit-decreasing pack of GPC graphs into <=T tiles of 128
    slots, <=C graphs each.  Returns (tile, col, offset) per graph."""
    order = np.argsort(-szs, kind="stable")
    used = []                       # [slots_used, ncols]
    gt = np.zeros(GPC, np.int32)
    gj = np.zeros(GPC, np.int32)
    go = np.zeros(GPC, np.int32)
    for g in order:
        s = int(szs[g])
        for ti in range(len(used)):
            if used[ti][0] + s <= 128 and used[ti][1] < C:
                break
        else:
            used.append([0, 0])
            ti = len(used) - 1
        gt[g] = ti
        gj[g] = used[ti][1]
        go[g] = used[ti][0]
        used[ti][0] += s
        used[ti][1] += 1
    assert len(used) <= T, f"pack needs {len(used)} tiles > {T}"
    return gt, gj, go


def _build_in_maps(inputs):
    import ml_dtypes
    bf = ml_dtypes.bfloat16
    f8 = ml_dtypes.float8_e4m3fn

    data = {}
    sz = {}
    for p, q in (("u", "up"), ("d", "down")):
        x = np.asarray(inputs[f"{q}_x"], np.float32)
        ei = np.asarray(inputs[f"{q}_edge_index"]).astype(np.int64)
        src, dst = ei[0], ei[1]
        valid = (dst % NPG) == (NPG - 1)
        cnt = np.bincount(src[valid], minlength=N).astype(np.float32)
        W = np.asarray(inputs[f"{q}_W"], np.float32)
        w_src = W @ np.asarray(inputs[f"{q}_att_src"], np.float32)
        w_dst = W @ np.asarray(inputs[f"{q}_att_dst"], np.float32)
        data[p] = dict(x=x, cnt=cnt, W=W, w_src=w_src, w_dst=w_dst,
                       bias=np.asarray(inputs[f"{q}_bias"], np.float32))
        sz[p] = (cnt.reshape(G, NPG) > 0).sum(1)
    mx = np.maximum(sz["u"], sz["d"])

    mlpW = np.asarray(inputs["mlp_W"], np.float32).reshape(H)
    cstf = np.zeros((128, 140), np.float32)
    cstf[0, 5] = float(np.asarray(inputs["mlp_b"]).reshape(-1)[0]) \
        + 0.25 * float(mlpW.sum())
    cstf[:, 6] = 0.5 * data["u"]["bias"]
    cstf[:, 7] = 0.5 * data["d"]["bias"]
    cstf[0, 8:136] = 1.0
    cstb = np.zeros((128, 261), np.float32)
    cstb[0:64, 0:128] = data["u"]["W"]
    cstb[0:64, 128:256] = data["d"]["W"]
    cstb[:, 256] = 0.25 * mlpW
    cstb[0:64, 257] = data["u"]["w_src"]
    cstb[64:128, 257] = data["u"]["w_dst"]
    cstb[0:64, 258] = data["d"]["w_src"]
    cstb[64:128, 258] = data["d"]["w_dst"]
    cstb[:, 260] = 1.0
    common = {"cstf": cstf, "cstb": cstb.astype(bf)}

    in_maps = []
    colmaps = []
    for cidx in range(NC):
        g0 = cidx * GPC
        gt, gj, go = _pack_core(mx[g0:g0 + GPC])
        m = dict(common)
        for p in ("u", "d"):
            b = data[p]
            cnt_c = b["cnt"][g0 * NPG:(g0 + GPC) * NPG]
            nzl = np.nonzero(cnt_c > 0)[0]          # local node idx
            gl = nzl // NPG                          # local graph
            rank = np.arange(len(nzl)) - np.searchsorted(nzl // NPG, gl)
            mm = go[gl] + rank
            tt = gt[gl]
            xrows = b["x"][g0 * NPG + nzl]           # [nnz, 64]
            xlast = b["x"][(g0 + gl) * NPG + NPG - 1]
            xg = np.zeros((128, T, 64), np.float32)
            xg[mm, tt, :] = xrows
            xgt2 = np.zeros((128, T, 128), np.float32)
            xgt2[:64, tt, mm] = xrows.T
            xgt2[64:, tt, mm] = xlast.T
            cm = np.zeros((128, T, C), np.float32)
            cm[mm, tt, gj[gl]] = cnt_c[nzl]
            cm8 = np.ascontiguousarray(
                cm.reshape(128, T, C).astype(f8)).view(np.uint16)
            chunks = []
            for c in range(2):
                ts = slice(c * HT, (c + 1) * HT)
                chunks.append(np.concatenate([
                    xgt2[:, ts].reshape(128, XGT2W).astype(bf),
                    xg[:, ts].reshape(128, XGW).astype(bf),
                    cm8[:, ts].reshape(128, CMB).view(bf)], axis=1))
            m[f"{p}_dat"] = np.ascontiguousarray(
                np.concatenate(chunks, axis=1))
        in_maps.append(m)
        colmaps.append(gt.astype(np.int64) * C + gj)
    return in_maps, colmaps


def _gather_out(results, colmaps):
    outs = []
    for r, cmap in zip(results, colmaps):
        o = np.asarray(r["out"], np.float32).reshape(TC)
        outs.append(o[cmap])
    return np.concatenate(outs).reshape(G, 1)


def kernel(**inputs):
    from concourse.bass_utils import run_bass_kernel_spmd

    _stash_obias(inputs)
    nc = _get_module()
    in_maps, colmaps = _build_in_maps(inputs)
    res = run_bass_kernel_spmd(nc, in_maps, core_ids=list(range(NC)))
    return _gather_out(res.results, colmaps)


def _stash_obias(inputs):
    mlpW = np.asarray(inputs["mlp_W"], np.float32).reshape(H)
    _CACHE["obias"] = float(np.asarray(inputs["mlp_b"]).reshape(-1)[0]) \
        + 0.25 * float(mlpW.sum())


# revision 13
# speedup vs baseline: 1.0861x; 1.0861x over previous
"""Trainium2 Bass kernel for nn_Event_Critic_Net (dual-branch GAT critic).

Math: the reference reads the GAT output only at the LAST node of each
graph (graphs are 32 contiguous nodes), so only edges with dst == last
node contribute.  Per graph g:

    out_g = sigmoid( (sum_n alpha[n] x[n]) @ W + bias )
    alpha[n] = cnt[n] exp(e[n]) / (sum + 1e-16)
    e[n] = leaky_relu(x[n].w_src + x[last(g)].w_dst)

Only nodes with cnt>0 matter (~7 of 32 per graph), so the host GATHERS
contributing nodes and bin-packs graphs into 128-slot tiles, T=36 tiles
x C=20 graph-columns per core per branch.  Per-slot logits come from a
single matmul per tile: the feature-major tensor stacks the slot's own
features (rows 0:64) on top of its graph's last-node features (rows
64:128), so lhsT.T @ [w_src; w_dst] yields e directly.  All scalar ops
(Prelu, Exp, Copy, Tanh) live in ONE activation table; sigmoid is
tanh-based with the algebra folded into the MLP head host-side:
    sig_u*sig_d*mlpW = mlpW/4 . (1 + tu + td + tu*td).
Denominators ride row 64 of the aggregation PSUM via a ones-column
matmul; their reciprocal happens AFTER a PE broadcast so it runs on 64
partitions.  Each branch chunk ships as ONE concatenated HWDGE DMA, u
before d on both queues.  Graphs are data-parallel over 8 cores.
"""

import numpy as np
from contextlib import ExitStack

NC = 8            # cores
N = 131072        # nodes total
G = 4096          # graphs
NPG = 32          # nodes per graph
S = 64            # state size
H = 128           # hidden size
GPC = G // NC     # 512 graphs per core
T = 36            # slot tiles per branch per core
C = 20            # graph columns per tile
TC = T * C        # 720 output columns per core
HT = T // 2       # tiles per chunk (2 chunks)
NEG = 0.2

XGT2W = HT * 128  # 2304
XGW = HT * 64     # 1152
CMW = HT * C      # 360 fp8 bytes -> 180 bf16 cols
CMB = CMW // 2    # 180
CHW = XGT2W + XGW + CMB  # 3636 cols per chunk

_CACHE = {}


def _build_module():
    import concourse.tile as tile
    from concourse import bacc, mybir
    from concourse.alu_op_type import AluOpType as Alu

    f32 = mybir.dt.float32
    bf16 = mybir.dt.bfloat16
    fp8 = mybir.dt.float8e4
    Act = mybir.ActivationFunctionType

    nc = bacc.Bacc("TRN2", target_bir_lowering=False, debug=False,
                   num_devices=NC)

    dram = {}
    for p in ("u", "d"):
        dram[p] = nc.dram_tensor(f"{p}_dat", [128, 2 * CHW], bf16,
                                 kind="ExternalInput")
    dram["cstf"] = nc.dram_tensor("cstf", [128, 72], f32,
                                  kind="ExternalInput")
    dram["cstb"] = nc.dram_tensor("cstb", [128, 261], bf16,
                                  kind="ExternalInput")
    out_dram = nc.dram_tensor("out", [1, TC], f32, kind="ExternalOutput")

    with tile.TileContext(nc) as tc, ExitStack() as ctx:
        const = ctx.enter_context(tc.tile_pool(name="const", bufs=1))
        xp = ctx.enter_context(tc.tile_pool(name="xp", bufs=2))
        wk = ctx.enter_context(tc.tile_pool(name="wk", bufs=2))
        pse = ctx.enter_context(tc.tile_pool(name="pse", bufs=1,
                                             space="PSUM"))
        psy = ctx.enter_context(tc.tile_pool(name="psy", bufs=2,
                                             space="PSUM"))
        psr = ctx.enter_context(tc.tile_pool(name="psr", bufs=2,
                                             space="PSUM"))
        psh = ctx.enter_context(tc.tile_pool(name="psh", bufs=2,
                                             space="PSUM"))

        cstb = const.tile([128, 261], bf16, tag="cstb")
        nc.scalar.dma_start(cstb[:], dram["cstb"].ap())
        cstf = const.tile([128, 72], f32, tag="cstf")
        nc.gpsimd.dma_start(cstf[:], dram["cstf"].ap())
        obias = cstf[0:1, 5:6]
        hbiases = {"u": cstf[:, 6:7], "d": cstf[:, 7:8]}   # bias/2
        ones64 = cstf[0:1, 8:72]
        Ws = {"u": cstb[0:64, 0:128], "d": cstb[0:64, 128:256]}
        mlpW4 = cstb[:, 256:257]                            # mlp_W/4
        w2s = {"u": cstb[:, 257:258], "d": cstb[:, 258:259]}
        ones128 = cstb[:, 260:261]

        st = {"u": {}, "d": {}}
        for p in ("u", "d"):
            st[p]["ch"] = []
            for c, eng in ((0, nc.sync), (1, nc.scalar)):
                t = xp.tile([128, CHW], bf16, tag=f"ch{c}",
                            name=f"ch{c}_{p}")
                eng.dma_start(t[:],
                              dram[p].ap()[:, c * CHW:(c + 1) * CHW])
                st[p]["ch"].append(t)

        eraw = pse.tile([128, 2 * T], f32, tag="eraw")
        tanh = {}

        def logits_masks_agg(p, c):
            """eraw -> prelu/exp -> mask -> agg + denom for chunk c."""
            s = st[p]
            off = (0 if p == "u" else T) + c * HT
            ch = s["ch"][c]
            for t in range(HT):
                nc.tensor.matmul(
                    eraw[:, off + t: off + t + 1],
                    ch[:, 128 * t:128 * t + 128],
                    w2s[p], start=True, stop=True)
            e = wk.tile([128, HT], f32, tag="e", name=f"e_{p}{c}")
            nc.scalar.activation(e[:], eraw[:, off:off + HT], Act.Prelu,
                                 alpha=NEG)
            ex = wk.tile([128, HT], f32, tag="ex", name=f"ex_{p}{c}")
            nc.scalar.activation(ex[:], e[:], Act.Exp)
            M = wk.tile([128, CMW], bf16, tag="M", name=f"M_{p}{c}")
            Mv = M[:].rearrange("p (t c) -> p t c", c=C)
            cmv = ch[:, XGT2W + XGW:CHW].bitcast(fp8).rearrange(
                "p (t c) -> p t c", c=C)
            exb = ex[:][:, :, None].broadcast_to([128, HT, C])
            nc.vector.tensor_tensor(Mv, exb, cmv, op=Alu.mult)
            ynT = psy.tile([128, CMW], f32, tag="ynT", name=f"ynT_{p}{c}")
            for t in range(HT):
                nc.tensor.matmul(
                    ynT[0:64, C * t:C * (t + 1)],
                    ch[:, XGT2W + 64 * t:XGT2W + 64 * t + 64],
                    M[:, C * t:C * (t + 1)], start=True, stop=True)
            nc.tensor.matmul(ynT[64:65, :], ones128, M[:],
                             start=True, stop=True)
            s.setdefault("ynT", []).append(ynT)

        def tail_half(p, c):
            """ysb(+eps) -> PE denom broadcast -> recip -> ynrm -> proj
            -> tanh for half c of branch p."""
            s = st[p]
            cs = slice(c * CMW, (c + 1) * CMW)
            ysb = s["ysb"]
            dnS = s["dnS"]
            nc.scalar.activation(dnS[:, cs], s["ynT"][c][64:65, :],
                                 Act.Copy, bias=1e-16)
            nc.scalar.activation(ysb[:, cs], s["ynT"][c][0:64, :],
                                 Act.Copy)
            rbc = psr.tile([64, CMW], f32, tag="rbc", name=f"rbc_{p}{c}")
            nc.tensor.matmul(rbc[:], ones64, dnS[:, cs],
                             start=True, stop=True)
            rin = wk.tile([64, CMW], f32, tag="rin", name=f"rin_{p}{c}")
            nc.vector.reciprocal_approx_fast(rin[:], rbc[:])
            ynrm = s["ynrm"]
            nc.vector.tensor_tensor(ynrm[:, cs], ysb[:, cs], rin[:],
                                    op=Alu.mult)
            hT = psh.tile([128, CMW], f32, tag="hT", name=f"hT_{p}{c}")
            nc.tensor.matmul(hT[:], Ws[p], ynrm[:, cs],
                             start=True, stop=True)
            nc.scalar.activation(tanh[p][:, cs], hT[:], Act.Tanh,
                                 bias=hbiases[p], scale=0.5)

        for p in ("u", "d"):
            st[p]["ysb"] = wk.tile([64, TC], f32, tag="ysb",
                                   name=f"ysb_{p}")
            st[p]["dnS"] = wk.tile([1, TC], f32, tag="dnS",
                                   name=f"dnS_{p}")
            st[p]["ynrm"] = wk.tile([64, TC], bf16, tag="ynrm",
                                    name=f"ynrm_{p}")
            tanh[p] = wk.tile([128, TC], bf16, tag="th", name=f"th_{p}")

        # program order: u fully, then d chunk-pipelined, tails woven in
        logits_masks_agg("u", 0)
        logits_masks_agg("u", 1)
        logits_masks_agg("d", 0)
        tail_half("u", 0)
        tail_half("u", 1)
        logits_masks_agg("d", 1)
        tail_half("d", 0)
        tail_half("d", 1)

        # ---- head: mlpW/4 . (1 + tu + td + tu*td), obias folded ----
        o_sb = wk.tile([1, TC], f32, tag="o_sb")
        for c in range(2):
            cs = slice(c * CMW, (c + 1) * CMW)
            tt = wk.tile([128, CMW], bf16, tag="tt", name=f"tt{c}")
            nc.vector.tensor_tensor(tt[:], tanh["u"][:, cs],
                                    tanh["d"][:, cs], op=Alu.mult)
            o_ps = pse.tile([1, CMW], f32, tag="mix", name=f"o{c}")
            nc.tensor.matmul(o_ps[:], mlpW4, tanh["u"][:, cs],
                             start=True, stop=False)
            nc.tensor.matmul(o_ps[:], mlpW4, tanh["d"][:, cs],
                             start=False, stop=False)
            nc.tensor.matmul(o_ps[:], mlpW4, tt[:],
                             start=False, stop=True)
            nc.vector.tensor_scalar(o_sb[:, cs], o_ps[:], obias, None,
                                    op0=Alu.add)
            nc.sync.dma_start(out_dram.ap()[:, cs], o_sb[:, cs])

    nc.compile()
    return nc


def _get_module():
    if "nc" not in _CACHE:
        _CACHE["nc"] = _build_module()
    return _CACHE["nc"]


def _pack_core(szs):
    """First-fit-decreasing pack of GPC graphs into <=T tiles of 128
    slots, <=C graphs each.  Returns (tile, col, offset) per graph."""
    order = np.argsort(-szs, kind="stable")
    used = []                       # [slots_used, ncols]
    gt = np.zeros(GPC, np.int32)
    gj = np.zeros(GPC, np.int32)
    go = np.zeros(GPC, np.int32)
    for g in order:
        s = int(szs[g])
        for ti in range(len(used)):
            if used[ti][0] + s <= 128 and used[ti][1] < C:
                break
        else:
            used.append([0, 0])
            ti = len(used) - 1
        gt[g] = ti
        gj[g] = used[ti][1]
        go[g] = used[ti][0]
        used[ti][0] += s
        used[ti][1] += 1
    assert len(used) <= T, f"pack needs {len(used)} tiles > {T}"
    return gt, gj, go


def _build_in_maps(inputs):
    import ml_dtypes
    bf = ml_dtypes.bfloat16
    f8 = ml_dtypes.float8_e4m3fn

    data = {}
    sz = {}
    for p, q in (("u", "up"), ("d", "down")):
        x = np.asarray(inputs[f"{q}_x"], np.float32)
        ei = np.asarray(inputs[f"{q}_edge_index"]).astype(np.int64)
        src, dst = ei[0], ei[1]
        valid = (dst % NPG) == (NPG - 1)
        cnt = np.bincount(src[valid], minlength=N).astype(np.float32)
        W = np.asarray(inputs[f"{q}_W"], np.float32)
        w_src = W @ np.asarray(inputs[f"{q}_att_src"], np.float32)
        w_dst = W @ np.asarray(inputs[f"{q}_att_dst"], np.float32)
        data[p] = dict(x=x, cnt=cnt, W=W, w_src=w_src, w_dst=w_dst,
                       bias=np.asarray(inputs[f"{q}_bias"], np.float32))
        sz[p] = (cnt.reshape(G, NPG) > 0).sum(1)
    mx = np.maximum(sz["u"], sz["d"])

    mlpW = np.asarray(inputs["mlp_W"], np.float32).reshape(H)
    cstf = np.zeros((128, 72), np.float32)
    cstf[0, 5] = float(np.asarray(inputs["mlp_b"]).reshape(-1)[0]) \
        + 0.25 * float(mlpW.sum())
    cstf[:, 6] = 0.5 * data["u"]["bias"]
    cstf[:, 7] = 0.5 * data["d"]["bias"]
    cstf[0, 8:72] = 1.0
    cstb = np.zeros((128, 261), np.float32)
    cstb[0:64, 0:128] = data["u"]["W"]
    cstb[0:64, 128:256] = data["d"]["W"]
    cstb[:, 256] = 0.25 * mlpW
    cstb[0:64, 257] = data["u"]["w_src"]
    cstb[64:128, 257] = data["u"]["w_dst"]
    cstb[0:64, 258] = data["d"]["w_src"]
    cstb[64:128, 258] = data["d"]["w_dst"]
    cstb[:, 260] = 1.0
    common = {"cstf": cstf, "cstb": cstb.astype(bf)}

    in_maps = []
    colmaps = []
    for cidx in range(NC):
        g0 = cidx * GPC
        gt, gj, go = _pack_core(mx[g0:g0 + GPC])
        m = dict(common)
        for p in ("u", "d"):
            b = data[p]
            cnt_c = b["cnt"][g0 * NPG:(g0 + GPC) * NPG]
            nzl = np.nonzero(cnt_c > 0)[0]          # local node idx
            gl = nzl // NPG                          # local graph
            rank = np.arange(len(nzl)) - np.searchsorted(nzl // NPG, gl)
            mm = go[gl] + rank
            tt = gt[gl]
            xrows = b["x"][g0 * NPG + nzl]           # [nnz, 64]
            xlast = b["x"][(g0 + gl) * NPG + NPG - 1]
            xg = np.zeros((128, T, 64), np.float32)
            xg[mm, tt, :] = xrows
            xgt2 = np.zeros((128, T, 128), np.float32)
            xgt2[:64, tt, mm] = xrows.T
            xgt2[64:, tt, mm] = xlast.T
            cm = np.zeros((128, T, C), np.float32)
            cm[mm, tt, gj[gl]] = cnt_c[nzl]
            cm8 = np.ascontiguousarray(cm.astype(f8)).view(np.uint16)
            chunks = []
            for c in range(2):
                ts = slice(c * HT, (c + 1) * HT)
                chunks.append(np.concatenate([
                    xgt2[:, ts].reshape(128, XGT2W).astype(bf),
                    xg[:, ts].reshape(128, XGW).astype(bf),
                    cm8[:, ts].reshape(128, CMB).view(bf)], axis=1))
            m[f"{p}_dat"] = np.ascontiguousarray(
                np.concatenate(chunks, axis=1))
        in_maps.append(m)
        colmaps.append(gt.astype(np.int64) * C + gj)
    return in_maps, colmaps


def _gather_out(results, colmaps):
    outs = []
    for r, cmap in zip(results, colmaps):
        o = np.asarray(r["out"], np.float32).reshape(TC)
        outs.append(o[cmap])
    return np.concatenate(outs).reshape(G, 1)


def kernel(**inputs):
    from concourse.bass_utils import run_bass_kernel_spmd

    nc = _get_module()
    in_maps, colmaps = _build_in_maps(inputs)
    res = run_bass_kernel_spmd(nc, in_maps, core_ids=list(range(NC)))
    return _gather_out(res.results, colmaps)
